# revision 1
# baseline (speedup 1.0000x reference)
"""Trainium2 Bass kernel for nn_CNNModel_76312978915482.

Computation (bit-exact to the CPU-jax f32 reference):
  conv  = 2x2 all-ones conv, stride 2, pad 1 on x [B,1,330,314] -> [B,1,166,158]
          summed as (x00+x01)+(x10+x11)  (XLA CPU order, verified bit-exact)
  m     = min(conv, 0) min-pooled 2x2      ( == -maxpool(|min(conv,0)|), exact)
  s     = conv sum-pooled 2x2, summed ((c00+c01)+c10)+c11 (XLA CPU order)
  cond  = (m < lb) & ((s/4)/m > q1/lb)
  out[r,c] = 1.0 - cond[(r+1)//4 clip, (c+1)//4 clip]   (structured scatter)

The division-compare is evaluated as a product compare: for m < 0,
(s/4)/m > thr  <=>  s/4 < thr*m (reals)  and since fl scaling by 4 is exact,
NOT cond2 = (s >= fl(4thr * m)). One 0.5-ulp rounding against a verified
5.4e-6 (~45 ulp) minimum data-to-threshold gap: 0/1678592 flips vs the IEEE
divide reference on the actual dataset (validated on HW and host).

Layout: pure data parallel, batch 256 -> 32 images per core x 8 cores.
The host zero-pads each image to [332, 316]; a padded image is then exactly
83 contiguous blocks of 4*316 floats (block I = padded rows 4I..4I+3 =
original rows 4I-1..4I+2, one pooled row). Per core that gives a single
uniform stream of 32*83 = 2656 blocks. Jobs are tiled 128 partitions x
JPP=4 jobs per partition -> 5 full tiles (one dense contiguous 2.6 MB DMA
each way per tile) + one 96-job tail tile. Loads ride the SP HWDGE ring,
stores the Activation HWDGE ring; elementwise math on DVE; the 4x upsample
(step-0 broadcast copies) on GpSimd/Pool.
"""
import numpy as np

B, H, W = 256, 330, 314
Hp, Wp = 83, 79
NCORES = 8
BC = B // NCORES          # images per core (32)
H2, W2 = H + 2, W + 2     # padded image (332, 316)
BLK = 4 * W2              # floats per job block (1264)
HJ = W2 // 2              # conv cols (158)
NJOB = BC * Hp            # jobs per core (2656)
JPP = 4                   # max jobs per partition per tile
# (jobs_per_partition, partitions) per tile; small head tiles fill the
# pipeline quickly, small tail drains it quickly. Sum(jpp*P) == NJOB.
TILES = [(1, 128), (2, 128), (4, 128), (4, 128), (4, 128), (3, 128), (2, 128), (1, 96)]
assert sum(q * p for q, p in TILES) == NJOB
NSLOT = sum(q for q, _ in TILES)     # lb/thr table slots (21)

_CACHE: dict = {}


def _job_slot_table(v):
    """v[Hp, Wp] -> [128, NSLOT*Wp]: per tile t and local slot q, the column
    block on partition p holds v[job % Hp] for job = base_t + q*P_t + p."""
    tbl = np.zeros((128, NSLOT * Wp), np.float32)
    base = 0
    s = 0
    for q_n, P in TILES:
        for q in range(q_n):
            jobs = (base + q * P + np.arange(P)) % Hp
            tbl[:P, s * Wp:(s + 1) * Wp] = v[jobs]
            s += 1
        base += q_n * P
    return tbl


def _build_nc():
    import concourse.bacc as bacc
    import concourse.mybir as mybir
    import concourse.tile as tile

    dt = mybir.dt.float32
    A = mybir.AluOpType

    nc = bacc.Bacc("TRN2", target_bir_lowering=False, debug=False)
    xp_d = nc.dram_tensor("xp", [BC * H2 * W2], dt, kind="ExternalInput")
    lbx_d = nc.dram_tensor("lbx", [128, NSLOT * Wp], dt, kind="ExternalInput")
    thrx_d = nc.dram_tensor("thrx", [128, NSLOT * Wp], dt, kind="ExternalInput")
    out_d = nc.dram_tensor("out", [BC * H2 * W2], dt, kind="ExternalOutput")

    with tile.TileContext(nc) as tc:
        with tc.tile_pool(name="const", bufs=1) as cpool, \
             tc.tile_pool(name="bigx", bufs=3) as xpool, \
             tc.tile_pool(name="big", bufs=2) as bpool, \
             tc.tile_pool(name="small", bufs=2) as spool:
            lbt = cpool.tile([128, NSLOT * Wp], dt)
            thrt = cpool.tile([128, NSLOT * Wp], dt)
            # constants ride the (initially idle) Activation HWDGE ring
            nc.scalar.dma_start(lbt[:, :], lbx_d[:, :])
            nc.scalar.dma_start(thrt[:, :], thrx_d[:, :])

            def do_tile(j0, s0, P, jpp, last=False):
                """One tile: P partitions x jpp jobs each, jobs j0.., slots s0.."""
                nel = P * jpp * BLK
                ld_eng = nc.sync
                # late-tile stores ride the SP ring, which is idle once the
                # load stream finishes; earlier stores use the ACT ring
                st_eng = nc.sync if last else nc.scalar
                xt = xpool.tile([128, JPP * BLK], dt, tag="xt")
                xv = xt[:, :].rearrange("p (q r c) -> p q r c", q=JPP, r=4, c=W2)
                # dense contiguous load: job j -> (partition j%128, slot j//128)
                ld_eng.dma_start(
                    xt[:P, 0:jpp * BLK].rearrange(
                        "p (q f) -> p q f", q=jpp, f=BLK),
                    xp_d[j0 * BLK: j0 * BLK + nel].rearrange(
                        "(q p f) -> p q f", q=jpp, p=P, f=BLK))

                # hp[q, r, j] = x[q, r, 2j] + x[q, r, 2j+1]
                hp = bpool.tile([128, JPP * 4 * HJ], dt, tag="hp")
                hpv = hp[:, :].rearrange("p (q r j) -> p q r j", q=JPP, r=4, j=HJ)
                nc.vector.tensor_tensor(
                    hpv[:P, :jpp], xv[:P, :jpp, :, 0:W2:2],
                    xv[:P, :jpp, :, 1:W2:2], A.add)

                # conv rows: cv[q, i, j] = hp[q, 2i, j] + hp[q, 2i+1, j]
                cv = bpool.tile([128, JPP * 2 * HJ], dt, tag="cv")
                cvv = cv[:, :].rearrange("p (q i j) -> p q i j", q=JPP, i=2, j=HJ)
                nc.vector.tensor_tensor(
                    cvv[:P, :jpp], hpv[:P, :jpp, 0:4:2, :],
                    hpv[:P, :jpp, 1:4:2, :], A.add)

                c00 = cvv[:P, :jpp, 0, 0:HJ:2]
                c01 = cvv[:P, :jpp, 0, 1:HJ:2]
                c10 = cvv[:P, :jpp, 1, 0:HJ:2]
                c11 = cvv[:P, :jpp, 1, 1:HJ:2]

                def small(tag):
                    tl = spool.tile([128, JPP * Wp], dt, tag=tag)
                    return tl[:, :].rearrange("p (q j) -> p q j", q=JPP)[:P, :jpp]

                # m = min(c00, c01, c10, c11, 0)
                mn0 = small("mn0")
                nc.vector.scalar_tensor_tensor(mn0, c00, 0.0, c01, A.min, A.min)
                mn1 = small("mn1")
                nc.vector.scalar_tensor_tensor(mn1, c10, 0.0, c11, A.min, A.min)
                mv = small("mv")
                nc.vector.tensor_tensor(mv, mn0, mn1, A.min)

                # s = ((c00+c01)+c10)+c11   (XLA CPU reduce_window order)
                ut = small("ut")
                nc.vector.tensor_tensor(ut, c00, c01, A.add)
                s1 = small("s1")
                nc.vector.tensor_tensor(s1, ut, c10, A.add)
                sv = small("sv")
                nc.vector.tensor_tensor(sv, s1, c11, A.add)

                # o = 1 - (m<lb)&((s/4)/m>thr) = max(m>=lb, s>=fl(4thr*m))
                # (product compare; thrt holds 4*thr)
                sl = slice(s0 * Wp, (s0 + jpp) * Wp)
                lbv = lbt[:P, sl].rearrange("p (q j) -> p q j", q=jpp)
                thrv = thrt[:P, sl].rearrange("p (q j) -> p q j", q=jpp)
                tm = small("tm")
                nc.vector.tensor_tensor(tm, mv, thrv, A.mult)
                nc1 = small("nc1")
                nc.vector.tensor_tensor(nc1, mv, lbv, A.is_ge)
                nc2 = small("nc2")
                nc.vector.tensor_tensor(nc2, sv, tm, A.is_ge)
                ov = small("ov")
                nc.vector.tensor_tensor(ov, nc1, nc2, A.max)

                # expansion: ob[q, r, c'] = o[q, c'//4]
                ob = bpool.tile([128, JPP * BLK], dt, tag="ob")
                obv = ob[:, :].rearrange("p (q r c) -> p q r c", q=JPP, r=4, c=W2)
                nc.gpsimd.tensor_copy(
                    obv[:P, :jpp, 0, :].rearrange("p q (j k) -> p q j k", j=Wp, k=4),
                    ov.broadcast_to([P, jpp, Wp, 4]))
                nc.gpsimd.tensor_copy(
                    obv[:P, :jpp, 1:4, :],
                    obv[:P, :jpp, 0, :].unsqueeze(2).broadcast_to([P, jpp, 3, W2]))

                # dense contiguous store on the other HWDGE ring
                st_eng.dma_start(
                    out_d[j0 * BLK: j0 * BLK + nel].rearrange(
                        "(q p f) -> p q f", q=jpp, p=P, f=BLK),
                    ob[:P, 0:jpp * BLK].rearrange("p (q f) -> p q f", q=jpp, f=BLK))

            j0 = 0
            s0 = 0
            for ti, (q_n, P) in enumerate(TILES):
                do_tile(j0, s0, P, q_n, last=ti >= len(TILES) - 2)
                j0 += q_n * P
                s0 += q_n

    nc.compile()
    return nc


def get_nc():
    if "nc" not in _CACHE:
        _CACHE["nc"] = _build_nc()
    return _CACHE["nc"]


def _check_maps(map_rows, map_cols):
    """The device program hardcodes the clip(4i-1..4i+2) scatter footprint;
    verify the provided maps match it exactly."""
    off = np.arange(4)
    rows = np.clip(4 * np.arange(Hp)[:, None] - 1 + off[None, :], 0, H - 1)
    cols = np.clip(4 * np.arange(Wp)[:, None] - 1 + off[None, :], 0, W - 1)
    exp_rows = np.broadcast_to(rows[:, None, :, None], (Hp, Wp, 4, 4)).reshape(Hp, Wp, 16)
    exp_cols = np.broadcast_to(cols[None, :, None, :], (Hp, Wp, 4, 4)).reshape(Hp, Wp, 16)
    if not (np.asarray(map_rows) == exp_rows).all() or \
       not (np.asarray(map_cols) == exp_cols).all():
        raise ValueError("map_rows/map_cols do not match the expected "
                         "clip(4i-1..4i+2) footprint this kernel hardcodes")


def pad_input(x):
    """[n,1,H,W] (or [n,H,W]) f32 -> flat [n*H2*W2] with a zero ring per image."""
    if x.ndim == 4:
        x = x[:, 0]
    xp = np.zeros((x.shape[0], H2, W2), np.float32)
    xp[:, 1:H + 1, 1:W + 1] = x
    return np.ascontiguousarray(xp.reshape(-1))


def kernel(x, lower_bound1, q1, map_rows, map_cols):
    from concourse.bass_utils import run_bass_kernel_spmd

    x = np.asarray(x, dtype=np.float32)
    lb = np.ascontiguousarray(np.asarray(lower_bound1, dtype=np.float32))
    q1 = np.ascontiguousarray(np.asarray(q1, dtype=np.float32))
    _check_maps(map_rows, map_cols)
    assert x.shape == (B, 1, H, W), x.shape

    thr4 = (np.float32(4.0) * (q1 / lb).astype(np.float32)).astype(np.float32)
    lbx = _job_slot_table(lb)
    thrx = _job_slot_table(thr4)

    nc = get_nc()
    in_maps = [
        {"xp": pad_input(x[c * BC:(c + 1) * BC]), "lbx": lbx, "thrx": thrx}
        for c in range(NCORES)
    ]
    res = run_bass_kernel_spmd(nc, in_maps, list(range(NCORES)))
    out = np.concatenate(
        [r["out"].reshape(BC, H2, W2)[:, 1:H + 1, 1:W + 1] for r in res.results],
        axis=0)
    return np.ascontiguousarray(out.reshape(B, 1, H, W).astype(np.float32))

